# revision 8
# baseline (speedup 1.0000x reference)
"""Multi-LoRA batched einsum kernel for Trainium2 (8 NeuronCores).

Computes: out[b,s,r] = sum_h x[b,s,h] * weight[adapter_ids[b], r, h]
  x:       [8, 2048, 8192] f32
  weight:  [1024, 16, 8192] f32   (adapter pool)
  adapter_ids: [8] i32
  out:     [8, 2048, 16] f32

This problem is pure HBM streaming (x is 512 MiB, output 1 MiB); the
roofline is bytes-of-x / aggregate HBM bandwidth. The kernel quantizes x
to fp8 E3M4 on the host (1 byte/elem, measured end-to-end rel err
~1.4e-2 vs the 2e-2 gate) and keeps the LoRA weights in bf16, quartering
the HBM traffic vs fp32.

Distribution (tensor-parallel over the hidden dim, per the sharding hint):
  - core d receives the H-slice [d*1024, (d+1)*1024) of x, laid out
    partition-major [128, B*K*S] so any span of the stream is one
    contiguous per-partition DRAM run.
  - the 8 active adapters are gathered on the host (adapter_ids is host
    data) and uploaded pre-transposed as [h, r] bf16 stationary tiles.
  - matmuls are column-tiled: the 4 output strips of a batch run in the
    4 col-groups of the PE array concurrently (tile_position=(0,32n)),
    all accumulating in one PSUM bank ([128,512] = 4 strips x 16 rows).
  - the x stream is pre-issued in 1 MiB half-batch quanta (plus a final
    taper) on the sync queue only. Tile hands each HWDGE DMA one of 8
    completion-sem lanes round-robin; a load's lane-reuse wait targets a
    load 8+ positions earlier (long complete), so the queue never
    starves. The fine quanta keep completion receipts flowing every
    ~2.4us so the PE runs warm and never trails the stream.
  - all batch outputs accumulate in one SBUF tile; batches 0-6 store in
    a single 896 KiB DMA on the scalar queue, and the final batch's
    128 KiB store rides the by-then-idle sync queue so nothing queues
    behind it. The host sums the 8 partial contractions (allreduce
    equivalent) and restores the [B, S, R] layout.
"""

import numpy as np

B, S, H, R, POOL = 8, 2048, 8192, 16, 1024
NCORES = 8
HS = H // NCORES   # 1024: per-core hidden slice
K = HS // 128      # 8 contraction chunks of 128
NS = 4             # output column strips (one per PE col-group)
SW = S // NS       # 512 = one PSUM bank of fp32
KS = K * S         # per-batch elements per partition (16 KiB at 1B)
# x load plan: (start_batch_elem, n_elems) in units of per-partition
# elements within the flat [128, B*K*S] stream. Half-batch (1 MiB) quanta:
# big enough for DMA line rate (8 KiB/partition descriptors), small enough
# that completion receipts land every ~2.4us — the PE starts at ~11us and
# its idle gaps stay under HAM's 3.4us re-throttle window, so the matmul
# chain runs warm and finishes with the stream instead of trailing it
# (coarse 4 MiB quanta measured 6+us of cold-PE tail past the last byte).
# The final batch tapers so the last load is 256 KiB.
LOADS = [(b * KS + h * 4 * S, 4 * S) for b in range(B - 1) for h in (0, 1)] + [
    (7 * KS, 4 * S),          # batch 7 k0-3  (1 MiB)
    (7 * KS + 4 * S, 2 * S),  # batch 7 k4-5  (512 KiB)
    (7 * KS + 6 * S, 1 * S),  # batch 7 k6    (256 KiB)
    (7 * KS + 7 * S, 1 * S),  # batch 7 k7    (256 KiB)
]

_cache: dict = {}


def _build():
    import concourse.mybir as mybir
    import concourse.tile as tile
    from concourse import bacc

    f32 = mybir.dt.float32
    bf16 = mybir.dt.bfloat16
    f8 = mybir.dt.float8e3

    nc = bacc.Bacc("TRN2", target_bir_lowering=False)
    # x stream, partition-major: x[b, s, d*1024 + k*128 + p] lives at
    # xq[p, (b*K + k)*S + s]
    xq = nc.dram_tensor("xq", [128, B * KS], f8, kind="ExternalInput")
    # host-gathered stationary tiles: wt[:, k, b, :] = [128, 16] for (b, k)
    wt = nc.dram_tensor("wt", [128, K, B, R], bf16, kind="ExternalInput")
    # outputs packed per strip with junk rows: outd[32n + r, b*SW + c] is
    # out[b, r, n*SW + c] for r < 16; host strips rows 16-31 of each group
    outd = nc.dram_tensor("outd", [128, B * SW], bf16, kind="ExternalOutput")

    with tile.TileContext(nc) as tc:
        import contextlib

        with contextlib.ExitStack() as stack:
            cpool = stack.enter_context(tc.tile_pool(name="const", bufs=1))
            # one pool per load size, exact buffer counts so no slot is
            # ever reused (reuse would add WAR waits on the load stream)
            xp1 = stack.enter_context(tc.tile_pool(name="x1m", bufs=15))
            xph = stack.enter_context(tc.tile_pool(name="xhm", bufs=1))
            xpq = stack.enter_context(tc.tile_pool(name="xqm", bufs=2))
            mps = stack.enter_context(
                tc.tile_pool(name="mps", bufs=2, space="PSUM")
            )
            osb = stack.enter_context(tc.tile_pool(name="osb", bufs=1))

            # wT on the scalar HWDGE queue; the sync queue carries only
            # the x stream
            wT = cpool.tile([128, K, B, R], bf16, name="wT")
            nc.scalar.dma_start(wT[:], wt[:])

            # all batch outputs collect here; one big store at the end
            o_all = osb.tile([128, B * SW], bf16, name="o_all")

            # pre-issue the entire x stream: 8 loads, 8 fresh sem lanes
            pool_of = {4 * S: (xp1, "t1"), 2 * S: (xph, "th"),
                       S: (xpq, "tq")}
            xt = []
            for i, (off, n) in enumerate(LOADS):
                pool, tag = pool_of[n]
                t = pool.tile([128, n], f8, tag=tag, name=f"xt_{i}")
                nc.sync.dma_start(t[:], xq[:, off:off + n])
                xt.append((off, n, t))

            def rhs_slice(b, k, n):
                """[128, SW] fp8 view of batch b, k-chunk k, strip n."""
                pos = (b * K + k) * S + n * SW
                for off, sz, t in xt:
                    if off <= pos and pos + SW <= off + sz:
                        return t[:, pos - off:pos - off + SW]
                raise AssertionError((b, k, n))

            for b in range(B):
                ps_b = mps.tile([128, SW], f32, tag="mm", name=f"mm_{b}")
                for k in range(K):
                    for n in range(NS):
                        nc.tensor.matmul(
                            ps_b[32 * n:32 * n + R, :],
                            lhsT=wT[:, k, b, :],
                            rhs=rhs_slice(b, k, n),
                            start=(k == 0),
                            stop=(k == K - 1),
                            tile_position=(0, 32 * n),
                        )
                dst = o_all[:, b * SW:(b + 1) * SW]
                if b < B - 1:
                    nc.vector.tensor_copy(dst, ps_b[:])
                else:
                    # final batch is the critical tail: one DVE copy feeding
                    # one store on the scalar queue (empty after wT; two
                    # half-stores measured slower — the second store's issue
                    # serializes on the sequencer and its HBM write receipt
                    # then gates the kernel-end barrier ~0.4us later)
                    nc.vector.tensor_copy(dst, ps_b[:])
                    nc.scalar.dma_start(
                        outd[:, b * SW:(b + 1) * SW], dst
                    )
                if b == B - 2:
                    # batches 0-6 store as one DMA on the SYNC queue: its
                    # descriptors enter the ring behind the x loads, so the
                    # per-queue FIFO guarantees every x byte (incl. the final
                    # taper) lands before any store byte moves — the store's
                    # 896 KiB then drains under the final batch's compute
                    # instead of delaying the x tail
                    nc.sync.dma_start(
                        outd[:, :(B - 1) * SW], o_all[:, :(B - 1) * SW]
                    )
    nc.compile()
    return nc


def _get_nc():
    if "nc" not in _cache:
        _cache["nc"] = _build()
    return _cache["nc"]


def _shard_inputs(x, weight, adapter_ids):
    """Host-side sharding: H-slice per core, contraction dim onto partitions,
    x quantized to fp8 e3m4, adapters gathered + transposed to bf16."""
    import ml_dtypes

    x = np.asarray(x, dtype=np.float32)
    weight = np.asarray(weight, dtype=np.float32)
    ids = np.asarray(adapter_ids).astype(np.int64)

    # quantize first (contiguous 512 MiB), then permute 1-byte data:
    # [NCORES, 128, B, K, S] with x[b, s, d*1024 + k*128 + p] = qr[d][p,b,k,s]
    q = np.ascontiguousarray(x).astype(ml_dtypes.float8_e3m4)
    qr = q.reshape(B, S, NCORES, K, 128).transpose(2, 4, 0, 3, 1)

    # gather + transpose the active adapters: wg[b, r, h] ->
    # wt[d][p, k, b, r] with h = d*1024 + k*128 + p
    wg = weight[ids]                                   # [B, R, H]
    wtT = (
        wg.reshape(B, R, NCORES, K, 128)
        .transpose(2, 4, 3, 0, 1)                      # [NC, 128, K, B, R]
        .astype(ml_dtypes.bfloat16)
    )

    return [
        {
            "xq": np.ascontiguousarray(qr[d]).reshape(128, B * KS),
            "wt": np.ascontiguousarray(wtT[d]),
        }
        for d in range(NCORES)
    ]


def _ensure_ntff_hook():
    """The container's antenv stub lacks axon_hooks, which
    run_bass_kernel_spmd imports whenever tracing is requested (including
    via the BASS_TRACE env var). Provide the module, and install the
    ctypes NTFF profile hook when the axon .so supports it."""
    import sys
    import types

    if "antenv.axon_hooks" in sys.modules:
        return
    mod = types.ModuleType("antenv.axon_hooks")
    holder = {"hook": None}
    mod.set_axon_ntff_profile_hook = lambda h: holder.__setitem__("hook", h)
    mod.get_axon_ntff_profile_hook = lambda: holder["hook"]
    sys.modules["antenv.axon_hooks"] = mod
    try:
        import antenv

        antenv.axon_hooks = mod
    except Exception:
        pass
    try:
        from trn_agent_boot.trn_boot import _ntff_profile_via_ctypes

        mod.set_axon_ntff_profile_hook(
            _ntff_profile_via_ctypes("/opt/axon/libaxon_pjrt.so")
        )
    except Exception:
        pass  # hookless: run_bass_kernel_spmd skips tracing gracefully


def _run(x, weight, adapter_ids, trace=False, trace_cores=None):
    from concourse.bass_utils import run_bass_kernel_spmd

    _ensure_ntff_hook()
    nc = _get_nc()
    in_maps = _shard_inputs(x, weight, adapter_ids)
    res = None
    for attempt in range(3):
        try:
            res = run_bass_kernel_spmd(
                nc,
                in_maps,
                core_ids=list(range(NCORES)),
                trace=trace,
                trace_cores=trace_cores,
            )
            break
        except Exception:
            # transient device wedges (e.g. NRT_EXEC_UNIT_UNRECOVERABLE)
            # clear on retry; re-raise if persistent
            if attempt == 2:
                raise
    # Host unshard: sum the 8 partial contractions, drop the junk rows of
    # each 32-partition col-group, restore [B, S, R]
    acc = np.zeros((128, B * SW), dtype=np.float32)
    for r in res.results:
        acc += r["outd"].astype(np.float32)
    # [4, 32, B, SW] -> valid rows -> [B, R, NS, SW] -> [B, S, R]
    full = acc.reshape(NS, 32, B, SW)[:, :R].transpose(2, 1, 0, 3)
    out = np.ascontiguousarray(
        full.reshape(B, R, S).transpose(0, 2, 1).astype(np.float32)
    )
    return out, res


def kernel(x, weight, weight_active, adapter_ids):
    # weight_active is all-zeros scratch fully overwritten by the reference's
    # dynamic_update_slice; it does not affect the output.
    out, _ = _run(x, weight, adapter_ids, trace=False)
    return out


# revision 11
# speedup vs baseline: 1.1498x; 1.1498x over previous
"""Multi-LoRA batched einsum kernel for Trainium2 (8 NeuronCores).

Computes: out[b,s,r] = sum_h x[b,s,h] * weight[adapter_ids[b], r, h]
  x:       [8, 2048, 8192] f32
  weight:  [1024, 16, 8192] f32   (adapter pool)
  adapter_ids: [8] i32
  out:     [8, 2048, 16] f32

This problem is pure HBM streaming (x is 512 MiB, output 1 MiB); the
roofline is bytes-of-x / aggregate HBM bandwidth. The kernel quantizes x
to fp8 E3M4 on the host (1 byte/elem, measured end-to-end rel err
~1.4e-2 vs the 2e-2 gate) and keeps the LoRA weights in bf16, quartering
the HBM traffic vs fp32.

Distribution (tensor-parallel over the hidden dim, per the sharding hint):
  - core d receives the H-slice [d*1024, (d+1)*1024) of x, laid out
    partition-major [128, B*K*S] so any span of the stream is one
    contiguous per-partition DRAM run.
  - the 8 active adapters are gathered on the host (adapter_ids is host
    data) and uploaded pre-transposed as [h, r] bf16 stationary tiles.
  - matmuls are column-tiled: the 4 output strips of a batch run in the
    4 col-groups of the PE array concurrently (tile_position=(0,32n)),
    all accumulating in one PSUM bank ([128,512] = 4 strips x 16 rows).
  - the x stream is pre-issued in 1 MiB half-batch quanta (plus a final
    taper) on the sync queue only. Tile hands each HWDGE DMA one of 8
    completion-sem lanes round-robin; a load's lane-reuse wait targets a
    load 8+ positions earlier (long complete), so the queue never
    starves. The fine quanta keep completion receipts flowing every
    ~2.4us so the PE runs warm and never trails the stream.
  - all batch outputs accumulate in one SBUF tile; batches 0-6 store in
    a single 896 KiB DMA FIFO'd at the end of the sync queue (so its
    bytes drain after the last x byte, hidden under the final batch's
    compute), and the final batch's 128 KiB store rides the scalar
    queue. The host sums the 8 partial contractions (allreduce
    equivalent) and restores the [B, S, R] layout.
"""

import numpy as np

B, S, H, R, POOL = 8, 2048, 8192, 16, 1024
NCORES = 8
HS = H // NCORES   # 1024: per-core hidden slice
K = HS // 128      # 8 contraction chunks of 128
NS = 4             # output column strips (one per PE col-group)
SW = S // NS       # 512 = one PSUM bank of fp32
KS = K * S         # per-batch elements per partition (16 KiB at 1B)
# x load plan: (start_batch_elem, n_elems) in units of per-partition
# elements within the flat [128, B*K*S] stream. Half-batch (1 MiB) quanta:
# big enough for DMA line rate (8 KiB/partition descriptors), small enough
# that completion receipts land every ~2.4us — the PE starts at ~11us and
# its idle gaps stay under HAM's 3.4us re-throttle window, so the matmul
# chain runs warm and finishes with the stream instead of trailing it
# (coarse 4 MiB quanta measured 6+us of cold-PE tail past the last byte).
# The final batch tapers so the last sync-queue load is 256 KiB. The very
# last k-chunk (b7 k7) instead rides the scalar queue, which is empty after
# wT: its 256 KiB lands by ~13us, so the final matmuls read long-resident
# data and the kernel tail never waits on a completion receipt for them —
# the sync stream's last receipt (k6) pipelines into the k6 matmuls while
# k7's follow back-to-back.
LOADS = [(b * KS + h * 4 * S, 4 * S) for b in range(B - 1) for h in (0, 1)] + [
    (7 * KS, 4 * S),          # batch 7 k0-3  (1 MiB)
    (7 * KS + 4 * S, 2 * S),  # batch 7 k4-5  (512 KiB)
    (7 * KS + 6 * S, 1 * S),  # batch 7 k6    (256 KiB)
]
LOAD_K7 = (7 * KS + 7 * S, 1 * S)  # batch 7 k7 (256 KiB, scalar queue)

_cache: dict = {}


def _build():
    import concourse.mybir as mybir
    import concourse.tile as tile
    from concourse import bacc

    f32 = mybir.dt.float32
    bf16 = mybir.dt.bfloat16
    f8 = mybir.dt.float8e3

    nc = bacc.Bacc("TRN2", target_bir_lowering=False)
    # x stream, partition-major: x[b, s, d*1024 + k*128 + p] lives at
    # xq[p, (b*K + k)*S + s]
    xq = nc.dram_tensor("xq", [128, B * KS], f8, kind="ExternalInput")
    # host-gathered stationary tiles: wt[:, k, b, :] = [128, 16] for (b, k)
    wt = nc.dram_tensor("wt", [128, K, B, R], bf16, kind="ExternalInput")
    # outputs packed per strip with junk rows: outd[32n + r, b*SW + c] is
    # out[b, r, n*SW + c] for r < 16; host strips rows 16-31 of each group
    outd = nc.dram_tensor("outd", [128, B * SW], bf16, kind="ExternalOutput")

    with tile.TileContext(nc) as tc:
        import contextlib

        with contextlib.ExitStack() as stack:
            cpool = stack.enter_context(tc.tile_pool(name="const", bufs=1))
            # one pool per load size, exact buffer counts so no slot is
            # ever reused (reuse would add WAR waits on the load stream)
            xp1 = stack.enter_context(tc.tile_pool(name="x1m", bufs=15))
            xph = stack.enter_context(tc.tile_pool(name="xhm", bufs=1))
            xpq = stack.enter_context(tc.tile_pool(name="xqm", bufs=2))
            mps = stack.enter_context(
                tc.tile_pool(name="mps", bufs=2, space="PSUM")
            )
            osb = stack.enter_context(tc.tile_pool(name="osb", bufs=1))

            # wT on the scalar HWDGE queue; the sync queue carries only
            # the x stream
            wT = cpool.tile([128, K, B, R], bf16, name="wT")
            nc.scalar.dma_start(wT[:], wt[:])

            # all batch outputs collect here; one big store at the end
            o_all = osb.tile([128, B * SW], bf16, name="o_all")

            # pre-issue the entire x stream
            pool_of = {4 * S: (xp1, "t1"), 2 * S: (xph, "th"),
                       S: (xpq, "tq")}
            xt = []
            for i, (off, n) in enumerate(LOADS):
                pool, tag = pool_of[n]
                t = pool.tile([128, n], f8, tag=tag, name=f"xt_{i}")
                nc.sync.dma_start(t[:], xq[:, off:off + n])
                xt.append((off, n, t))
            # b7 k7 on the scalar queue: drains right after wT, ~37us
            # before it is needed
            off, n = LOAD_K7
            t = xpq.tile([128, n], f8, tag="tq", name="xt_k7")
            nc.scalar.dma_start(t[:], xq[:, off:off + n])
            xt.append((off, n, t))

            def rhs_slice(b, k, n):
                """[128, SW] fp8 view of batch b, k-chunk k, strip n."""
                pos = (b * K + k) * S + n * SW
                for off, sz, t in xt:
                    if off <= pos and pos + SW <= off + sz:
                        return t[:, pos - off:pos - off + SW]
                raise AssertionError((b, k, n))

            for b in range(B):
                ps_b = mps.tile([128, SW], f32, tag="mm", name=f"mm_{b}")
                for k in range(K):
                    for n in range(NS):
                        nc.tensor.matmul(
                            ps_b[32 * n:32 * n + R, :],
                            lhsT=wT[:, k, b, :],
                            rhs=rhs_slice(b, k, n),
                            start=(k == 0),
                            stop=(k == K - 1),
                            tile_position=(0, 32 * n),
                        )
                dst = o_all[:, b * SW:(b + 1) * SW]
                if b < B - 1:
                    nc.vector.tensor_copy(dst, ps_b[:])
                else:
                    # final batch is the critical tail: one DVE copy feeding
                    # one store on the scalar queue (empty after wT; two
                    # half-stores measured slower — the second store's issue
                    # serializes on the sequencer and its HBM write receipt
                    # then gates the kernel-end barrier ~0.4us later)
                    nc.vector.tensor_copy(dst, ps_b[:])
                    nc.scalar.dma_start(
                        outd[:, b * SW:(b + 1) * SW], dst
                    )
                if b == B - 2:
                    # batches 0-6 store as one DMA on the SYNC queue: its
                    # descriptors enter the ring behind the x loads, so the
                    # per-queue FIFO guarantees every x byte (incl. the final
                    # taper) lands before any store byte moves — the store's
                    # 896 KiB then drains under the final batch's compute
                    # instead of delaying the x tail
                    nc.sync.dma_start(
                        outd[:, :(B - 1) * SW], o_all[:, :(B - 1) * SW]
                    )
    nc.compile()
    return nc


def _get_nc():
    if "nc" not in _cache:
        _cache["nc"] = _build()
    return _cache["nc"]


def _shard_inputs(x, weight, adapter_ids):
    """Host-side sharding: H-slice per core, contraction dim onto partitions,
    x quantized to fp8 e3m4, adapters gathered + transposed to bf16."""
    import ml_dtypes

    x = np.asarray(x, dtype=np.float32)
    weight = np.asarray(weight, dtype=np.float32)
    ids = np.asarray(adapter_ids).astype(np.int64)

    # quantize first (contiguous 512 MiB), then permute 1-byte data:
    # [NCORES, 128, B, K, S] with x[b, s, d*1024 + k*128 + p] = qr[d][p,b,k,s]
    q = np.ascontiguousarray(x).astype(ml_dtypes.float8_e3m4)
    qr = q.reshape(B, S, NCORES, K, 128).transpose(2, 4, 0, 3, 1)

    # gather + transpose the active adapters: wg[b, r, h] ->
    # wt[d][p, k, b, r] with h = d*1024 + k*128 + p
    wg = weight[ids]                                   # [B, R, H]
    wtT = (
        wg.reshape(B, R, NCORES, K, 128)
        .transpose(2, 4, 3, 0, 1)                      # [NC, 128, K, B, R]
        .astype(ml_dtypes.bfloat16)
    )

    return [
        {
            "xq": np.ascontiguousarray(qr[d]).reshape(128, B * KS),
            "wt": np.ascontiguousarray(wtT[d]),
        }
        for d in range(NCORES)
    ]


def _ensure_ntff_hook():
    """The container's antenv stub lacks axon_hooks, which
    run_bass_kernel_spmd imports whenever tracing is requested (including
    via the BASS_TRACE env var). Provide the module, and install the
    ctypes NTFF profile hook when the axon .so supports it."""
    import sys
    import types

    if "antenv.axon_hooks" in sys.modules:
        return
    mod = types.ModuleType("antenv.axon_hooks")
    holder = {"hook": None}
    mod.set_axon_ntff_profile_hook = lambda h: holder.__setitem__("hook", h)
    mod.get_axon_ntff_profile_hook = lambda: holder["hook"]
    sys.modules["antenv.axon_hooks"] = mod
    try:
        import antenv

        antenv.axon_hooks = mod
    except Exception:
        pass
    try:
        from trn_agent_boot.trn_boot import _ntff_profile_via_ctypes

        mod.set_axon_ntff_profile_hook(
            _ntff_profile_via_ctypes("/opt/axon/libaxon_pjrt.so")
        )
    except Exception:
        pass  # hookless: run_bass_kernel_spmd skips tracing gracefully


def _run(x, weight, adapter_ids, trace=False, trace_cores=None):
    from concourse.bass_utils import run_bass_kernel_spmd

    _ensure_ntff_hook()
    nc = _get_nc()
    in_maps = _shard_inputs(x, weight, adapter_ids)
    res = None
    for attempt in range(3):
        try:
            res = run_bass_kernel_spmd(
                nc,
                in_maps,
                core_ids=list(range(NCORES)),
                trace=trace,
                trace_cores=trace_cores,
            )
            break
        except Exception:
            # transient device wedges (e.g. NRT_EXEC_UNIT_UNRECOVERABLE)
            # clear on retry; re-raise if persistent
            if attempt == 2:
                raise
    # Host unshard: sum the 8 partial contractions, drop the junk rows of
    # each 32-partition col-group, restore [B, S, R]
    acc = np.zeros((128, B * SW), dtype=np.float32)
    for r in res.results:
        acc += r["outd"].astype(np.float32)
    # [4, 32, B, SW] -> valid rows -> [B, R, NS, SW] -> [B, S, R]
    full = acc.reshape(NS, 32, B, SW)[:, :R].transpose(2, 1, 0, 3)
    out = np.ascontiguousarray(
        full.reshape(B, R, S).transpose(0, 2, 1).astype(np.float32)
    )
    return out, res


def kernel(x, weight, weight_active, adapter_ids):
    # weight_active is all-zeros scratch fully overwritten by the reference's
    # dynamic_update_slice; it does not affect the output.
    out, _ = _run(x, weight, adapter_ids, trace=False)
    return out
